# revision 3
# baseline (speedup 1.0000x reference)
"""BitNetLinear forward on 8 TRN2 NeuronCores.

out = x @ (alpha * clip(round(W/alpha), -1, 1))^T
  x [4, 2048, 4096] f32, W [4096, 4096] f32, alpha scalar f32.

Strategy: data-parallel over the 8192 x-rows (1024 rows/core), W replicated.
No collectives. Host side only reshapes/slices (layout); all arithmetic
(ternary quantization + matmul + alpha scaling) runs on device.

Device kernel (per core), v2 — ldweights-amortized matmul structure:
  - x^T shard resident in SBUF as bf16 [128, 8mo, 32k, 128m] (64KB/part).
  - W^T streamed in 8 panels of [8ko, 2048n] (nh-half x kb-block),
    double-buffered; ternarized on the fly to {-2,0,2} bf16 via
    T' = Sign(w + a/2) + Sign(w - a/2)  (2 ACT signs + 1 DVE add).
  - Matmuls: for each (panel, mo): 8 k x 4 n-tiles of FD=512, the 4
    consecutive MMs per k share one lhsT (x tile) -> the PE's LDWEIGHTS
    pull-ahead hides most of the stationary reload (146 ns/MM measured
    vs 194 ns/MM at ldw-per-1).
  - PSUM [128, 2048] (4 banks) per mo, 2 mo in flight; k-accumulation is
    split across 4 kb blocks: partials scaled by alpha/2 and accumulated
    in a bf16 SBUF acc [128, 8mo, 2048] via DVE tensor_scalar /
    scalar_tensor_tensor; last block writes f32 out.
"""

import contextlib
import sys

if "/opt/trn_rl_repo" not in sys.path:
    sys.path.insert(0, "/opt/trn_rl_repo")

import numpy as np

import concourse.bass as bass  # noqa: F401
import concourse.mybir as mybir
import concourse.tile as tile
from concourse import bacc
from concourse.bass_utils import run_bass_kernel_spmd

P = 128
N_CORES = 8
D_IN = 4096
D_OUT = 4096
M_TOT = 4 * 2048
M_SHARD = M_TOT // N_CORES  # 1024
KO = D_IN // P  # 32 k-tiles
MO = M_SHARD // P  # 8 m-tiles
NH = 2  # n halves
NW = D_OUT // NH  # 2048
KB = 4  # k blocks
KBO = KO // KB  # 8 ko per block

F32 = mybir.dt.float32
BF16 = mybir.dt.bfloat16
MULT = mybir.AluOpType.mult
ADD = mybir.AluOpType.add


def build(reps=1, mode="full"):
    """mode: 'full' (real kernel), 'pe' (probe: static operands, no input
    DMA/quant -> measures MM+evict pipeline)."""
    nc = bacc.Bacc("TRN2", target_bir_lowering=False, debug=False,
                   num_devices=N_CORES)
    xt_d = nc.declare_dram_parameter("xt", [P, MO, KO, P], F32, isOutput=False)
    wt_d = nc.declare_dram_parameter("wt", [P, KO, D_OUT], F32, isOutput=False)
    al_d = nc.declare_dram_parameter("alpha", [1, 1], F32, isOutput=False)
    out_d = nc.declare_dram_parameter("out", [P, MO, D_OUT], F32, isOutput=True)

    with tile.TileContext(nc) as tc:
        with (
            tc.tile_pool(name="const", bufs=1) as const,
            tc.tile_pool(name="xres", bufs=1) as xres_pool,
            tc.tile_pool(name="accp", bufs=1) as accp,
            tc.tile_pool(name="wq", bufs=2) as wqp,
            tc.tile_pool(name="stage", bufs=2) as stage,
            tc.tile_pool(name="s2", bufs=2) as s2p,
            tc.tile_pool(name="ot", bufs=2) as otp,
            tc.tile_pool(name="psum", bufs=2, space="PSUM") as psum,
        ):
            pe_init = {}
            if mode == "pe":
                pe_init["xres"] = xres_pool.tile([P, MO, KO, P], BF16,
                                                 tag="xres", name="xres")
                nc.vector.memset(pe_init["xres"][:, :, :, :], 0.25)
                pe_init["wq"] = wqp.tile([P, KBO, NW], BF16, tag="wq",
                                         name="wq_static")
                nc.vector.memset(pe_init["wq"][:, :, :], 0.25)

            rep_ctx = (
                tc.For_i(0, reps, 1) if reps > 1 else contextlib.nullcontext()
            )
            with rep_ctx:
                a1 = const.tile([1, 1], F32)
                nc.sync.dma_start(out=a1[:, :], in_=al_d.ap()[:, :])
                ab = const.tile([P, 1], F32)
                nc.gpsimd.partition_broadcast(ab[:, :], a1[:, :])
                half = const.tile([P, 1], F32)
                nc.vector.tensor_scalar_mul(half[:, :], ab[:, :], 0.5)
                neghalf = const.tile([P, 1], F32)
                nc.vector.tensor_scalar_mul(neghalf[:, :], ab[:, :], -0.5)

                acc = accp.tile([P, MO, NW], BF16, tag="acc", name="acc")

                if mode == "pe":
                    xres = pe_init["xres"]
                else:
                    xres = xres_pool.tile([P, MO, KO, P], BF16, tag="xres",
                                          name="xres")

                def load_x(mo, kh):
                    # x chunk [128, 16ko, 128m] f32 -> bf16 cast on GpSimd
                    st = stage.tile([P, KO // 2, P], F32, tag="stage",
                                    name="xstage")
                    nc.sync.dma_start(
                        out=st[:, :, :],
                        in_=xt_d.ap()[:, mo, kh * (KO // 2):(kh + 1) * (KO // 2), :],
                    )
                    nc.gpsimd.tensor_copy(
                        xres[:, mo, kh * (KO // 2):(kh + 1) * (KO // 2), :],
                        st[:, :, :],
                    )

                def new_panel():
                    return wqp.tile([P, KBO, NW], BF16, tag="wq", name="wq")

                def produce_slice(wq, nh, kb, j):
                    # one W k-slice [128, 2048] f32 -> ternary*2 bf16
                    k = kb * KBO + j
                    st = stage.tile([P, 1, NW], F32, tag="stage",
                                    name="wstage")
                    nc.sync.dma_start(
                        out=st[:, :, :],
                        in_=wt_d.ap()[:, k:k + 1, nh * NW:(nh + 1) * NW],
                    )
                    nc.scalar.sign(wq[:, j, :], st[:, 0, :], bias=half[:, :])
                    s2 = s2p.tile([P, NW], BF16, tag="s2", name="s2")
                    nc.scalar.sign(s2[:, :], st[:, 0, :], bias=neghalf[:, :])
                    nc.vector.tensor_tensor(wq[:, j, :], wq[:, j, :],
                                            s2[:, :], ADD)

                # ---- head: x mo0-3 + panel 0, interleaved in need order
                panels = {}
                if mode == "pe":
                    panels[0] = pe_init["wq"]
                else:
                    load_x(0, 0)
                    load_x(0, 1)
                    panels[0] = new_panel()
                    produce_slice(panels[0], 0, 0, 0)
                    produce_slice(panels[0], 0, 0, 1)
                    load_x(1, 0)
                    load_x(1, 1)
                    produce_slice(panels[0], 0, 0, 2)
                    produce_slice(panels[0], 0, 0, 3)
                    load_x(2, 0)
                    load_x(2, 1)
                    produce_slice(panels[0], 0, 0, 4)
                    produce_slice(panels[0], 0, 0, 5)
                    load_x(3, 0)
                    load_x(3, 1)
                    produce_slice(panels[0], 0, 0, 6)
                    produce_slice(panels[0], 0, 0, 7)

                for p in range(NH * KB):
                    nh, kb = divmod(p, KB)
                    wq = panels.pop(p)
                    np_nh, np_kb = divmod(p + 1, KB)
                    for mo in range(MO):
                        ps = psum.tile([P, NW], F32, tag="ps", name="ps")
                        for j in range(KBO):
                            k = kb * KBO + j
                            for t in range(NW // 512):
                                nc.tensor.matmul(
                                    ps[:, t * 512:(t + 1) * 512],
                                    lhsT=xres[:, mo, k, :],
                                    rhs=wq[:, j, t * 512:(t + 1) * 512],
                                    start=(j == 0),
                                    stop=(j == KBO - 1),
                                )
                        # eviction: acc += alpha/2 * psum (bf16), last kb
                        # writes f32 out.
                        if kb == 0:
                            nc.vector.tensor_scalar(
                                acc[:, mo, :], ps[:, :], half[:, :], None,
                                MULT,
                            )
                        elif kb < KB - 1:
                            nc.vector.scalar_tensor_tensor(
                                acc[:, mo, :], ps[:, :], half[:, :],
                                acc[:, mo, :], MULT, ADD,
                            )
                        else:
                            ot = otp.tile([P, NW], F32, tag="ot", name="ot")
                            nc.vector.scalar_tensor_tensor(
                                ot[:, :], ps[:, :], half[:, :],
                                acc[:, mo, :], MULT, ADD,
                            )
                            nc.scalar.dma_start(
                                out=out_d.ap()[:, mo, nh * NW:(nh + 1) * NW],
                                in_=ot[:, :],
                            )
                        # software-pipeline: x mo4-7 during panel 0; next
                        # panel's slices one per mo.
                        if mode != "pe":
                            if p == 0 and 0 <= mo < 4:
                                load_x(mo + 4, 0)
                                load_x(mo + 4, 1)
                            if p + 1 < NH * KB:
                                if p + 1 not in panels:
                                    panels[p + 1] = new_panel()
                                produce_slice(panels[p + 1], np_nh, np_kb, mo)
                        elif p + 1 < NH * KB:
                            panels[p + 1] = pe_init["wq"]

    nc.compile()
    return nc


_NC_CACHE = {}


def _get_nc():
    if "nc" not in _NC_CACHE:
        _NC_CACHE["nc"] = build()
    return _NC_CACHE["nc"]


def make_in_maps(x, W, alpha):
    x = np.ascontiguousarray(np.asarray(x, np.float32)).reshape(M_TOT, D_IN)
    W = np.ascontiguousarray(np.asarray(W, np.float32))
    a = np.full((1, 1), np.float32(np.asarray(alpha)), np.float32)
    # wt[p, k, n] = W[n, k*128 + p]
    wt = np.ascontiguousarray(W.reshape(D_OUT, KO, P).transpose(2, 1, 0))
    in_maps = []
    for c in range(N_CORES):
        xs = x[c * M_SHARD:(c + 1) * M_SHARD]
        # xt[p, mo, k, m] = xs[mo*128 + m, k*128 + p]
        xt = np.ascontiguousarray(
            xs.reshape(MO, P, KO, P).transpose(3, 0, 2, 1))
        in_maps.append({"xt": xt, "wt": wt, "alpha": a})
    return in_maps


def gather_out(results):
    outs = []
    for c in range(N_CORES):
        o = results[c]["out"]  # [P, MO, D_OUT]; row = mo*128 + p
        outs.append(o.transpose(1, 0, 2).reshape(M_SHARD, D_OUT))
    return np.concatenate(outs, axis=0).reshape(4, 2048, D_OUT)


def kernel(x, W, alpha):
    nc = _get_nc()
    in_maps = make_in_maps(x, W, alpha)
    res = run_bass_kernel_spmd(nc, in_maps, core_ids=list(range(N_CORES)))
    return gather_out(res.results)


# revision 4
# speedup vs baseline: 1.0640x; 1.0640x over previous
"""BitNetLinear forward on 8 TRN2 NeuronCores.

out = x @ (alpha * clip(round(W/alpha), -1, 1))^T
  x [4, 2048, 4096] f32, W [4096, 4096] f32, alpha scalar f32.

Strategy: data-parallel over the 8192 x-rows (1024 rows/core), W replicated.
No collectives. Host side only reshapes/slices (layout); all arithmetic
(ternary quantization + matmul + alpha scaling) runs on device.

Device kernel (per core), v3 — ldweights-amortized matmuls + balanced
engine assignment:
  - x^T shard resident in SBUF as bf16, 8 per-mo tiles [128, 32k, 128m]
    (whole-tile dep granularity: per-mo tiles let matmuls start as soon
    as their own mo's x is loaded). Cast f32->bf16 on GpSimd.
  - W^T streamed in 8 panels of [8ko, 2048n], double-buffered.
    Quantization per 1024-col half-slice, split across engines:
      s1 = Sign(w + a/2)            (ACT, bf16)
      g  = (w >= a/2)               (DVE tensor_scalar is_ge, bf16)
      wq = 2*g + s1                 (DVE scalar_tensor_tensor)
    wq encodes 2*ternary + 1; the +1 is removed at eviction via a
    per-partition bias  -a/2 * colsum(x)[m]  (colsum via FD=1 matmuls).
  - Matmuls: per (panel, mo): 8k x 4n FD=512, 4 consecutive MMs share
    one lhsT -> LDWEIGHTS amortized (measured ~111 ns/MM effective).
  - PSUM [128, 2048] (4 banks) per mo, 2 in flight; k split in 4 kb
    blocks, partials scaled by a/2 into bf16 acc via DVE tensor_scalar /
    scalar_tensor_tensor; kb0 adds the colsum bias; kb3 writes f32 out.
"""

import contextlib
import sys

if "/opt/trn_rl_repo" not in sys.path:
    sys.path.insert(0, "/opt/trn_rl_repo")

import numpy as np

import concourse.bass as bass  # noqa: F401
import concourse.mybir as mybir
import concourse.tile as tile
from concourse import bacc
from concourse.bass_utils import run_bass_kernel_spmd

P = 128
N_CORES = 8
D_IN = 4096
D_OUT = 4096
M_TOT = 4 * 2048
M_SHARD = M_TOT // N_CORES  # 1024
KO = D_IN // P  # 32 k-tiles
MO = M_SHARD // P  # 8 m-tiles
NH = 2  # n halves
NW = D_OUT // NH  # 2048
KB = 4  # k blocks
KBO = KO // KB  # 8 ko per block

F32 = mybir.dt.float32
BF16 = mybir.dt.bfloat16
MULT = mybir.AluOpType.mult
ADD = mybir.AluOpType.add
IS_GE = mybir.AluOpType.is_ge


def build(reps=1, mode="full", quant="split"):
    """mode: 'full' (real), 'pe' (static operands: MM+evict pipeline only).
    quant: 'split' (ACT sign + DVE is_ge + colsum bias) or 'act2'
    (2 ACT signs + DVE add, no bias)."""
    nc = bacc.Bacc("TRN2", target_bir_lowering=False, debug=False,
                   num_devices=N_CORES)
    xt_d = nc.declare_dram_parameter("xt", [P, MO, KO, P], F32, isOutput=False)
    wt_d = nc.declare_dram_parameter("wt", [P, KO, D_OUT], F32, isOutput=False)
    al_d = nc.declare_dram_parameter("alpha", [1, 1], F32, isOutput=False)
    out_d = nc.declare_dram_parameter("out", [P, MO, D_OUT], F32, isOutput=True)

    with tile.TileContext(nc) as tc:
        with (
            tc.tile_pool(name="const", bufs=1) as const,
            tc.tile_pool(name="xres", bufs=1) as xres_pool,
            tc.tile_pool(name="nas", bufs=1) as nas_pool,
            tc.tile_pool(name="accp", bufs=1) as accp,
            tc.tile_pool(name="wq", bufs=2) as wqp,
            tc.tile_pool(name="xstage", bufs=2) as xstage,
            tc.tile_pool(name="wstage", bufs=3) as wstage,
            tc.tile_pool(name="s2", bufs=2) as s2p,
            tc.tile_pool(name="ot", bufs=2) as otp,
            tc.tile_pool(name="psum", bufs=2, space="PSUM") as psum,
        ):
            pe_init = {}
            if mode == "pe":
                pe_init["xres"] = [
                    xres_pool.tile([P, KO, P], BF16, tag=f"xres{mo}",
                                   name=f"xres{mo}")
                    for mo in range(MO)
                ]
                for mo in range(MO):
                    nc.vector.memset(pe_init["xres"][mo][:, :, :], 0.25)
                pe_init["wq"] = wqp.tile([P, KBO, NW], BF16, tag="wq",
                                         name="wq_static")
                nc.vector.memset(pe_init["wq"][:, :, :], 0.25)

            rep_ctx = (
                tc.For_i(0, reps, 1) if reps > 1 else contextlib.nullcontext()
            )
            with rep_ctx:
                a1 = const.tile([1, 1], F32)
                nc.sync.dma_start(out=a1[:, :], in_=al_d.ap()[:, :])
                ab = const.tile([P, 1], F32)
                nc.gpsimd.partition_broadcast(ab[:, :], a1[:, :])
                half = const.tile([P, 1], F32)
                nc.vector.tensor_scalar_mul(half[:, :], ab[:, :], 0.5)
                neghalf = const.tile([P, 1], F32)
                nc.vector.tensor_scalar_mul(neghalf[:, :], ab[:, :], -0.5)
                ones = const.tile([P, 1], BF16)
                nc.vector.memset(ones[:, :], 1.0)

                acc = accp.tile([P, MO, NW], BF16, tag="acc", name="acc")

                if mode == "pe":
                    xres = pe_init["xres"]
                else:
                    xres = [
                        xres_pool.tile([P, KO, P], BF16, tag=f"xres{mo}",
                                       name=f"xres{mo}")
                        for mo in range(MO)
                    ]
                nas = [
                    nas_pool.tile([P, 1], F32, tag=f"nas{mo}", name=f"nas{mo}")
                    for mo in range(MO)
                ]

                def load_x(mo, kh):
                    st = xstage.tile([P, KO // 2, P], F32, tag="xstage",
                                     name="xstage")
                    nc.sync.dma_start(
                        out=st[:, :, :],
                        in_=xt_d.ap()[:, mo,
                                      kh * (KO // 2):(kh + 1) * (KO // 2), :],
                    )
                    nc.gpsimd.tensor_copy(
                        xres[mo][:, kh * (KO // 2):(kh + 1) * (KO // 2), :],
                        st[:, :, :],
                    )

                def colsum(mo):
                    # nas[mo] = -alpha/2 * sum_k x_bf16[k, m]  (per out-row m)
                    cs = psum.tile([P, 1], F32, tag="ps", name="cs")
                    for k in range(KO):
                        nc.tensor.matmul(cs[:, :], lhsT=xres[mo][:, k, :],
                                         rhs=ones[:, :], start=(k == 0),
                                         stop=(k == KO - 1))
                    nc.vector.tensor_scalar(nas[mo][:, :], cs[:, :],
                                            neghalf[:, :], None, MULT)

                def new_panel():
                    return wqp.tile([P, KBO, NW], BF16, tag="wq", name="wq")

                def produce_half(wq, nh, kb, j, h):
                    # one W half-slice [128, 1024] f32 -> encoded ternary bf16
                    k = kb * KBO + j
                    c0 = nh * NW + h * 1024
                    st = wstage.tile([P, 1, 1024], F32, tag="wstage",
                                     name="wstage")
                    nc.sync.dma_start(
                        out=st[:, :, :],
                        in_=wt_d.ap()[:, k:k + 1, c0:c0 + 1024],
                    )
                    dst = wq[:, j, h * 1024:(h + 1) * 1024]
                    if quant == "split":
                        # wq = 2*(w >= a/2) + sign(w + a/2)  (encodes 2t+1)
                        nc.scalar.sign(dst, st[:, 0, :], bias=half[:, :])
                        g = s2p.tile([P, 1024], BF16, tag="s2", name="g")
                        nc.vector.tensor_scalar(g[:, :], st[:, 0, :],
                                                half[:, :], None, IS_GE)
                        nc.vector.scalar_tensor_tensor(dst, g[:, :], 2.0,
                                                       dst, MULT, ADD)
                    else:
                        nc.scalar.sign(dst, st[:, 0, :], bias=half[:, :])
                        s2 = s2p.tile([P, 1024], BF16, tag="s2", name="s2")
                        nc.scalar.sign(s2[:, :], st[:, 0, :],
                                       bias=neghalf[:, :])
                        nc.vector.tensor_tensor(dst, dst, s2[:, :], ADD)

                # ---- head: x mo0-3 + colsums + panel 0, in need order
                panels = {}
                if mode == "pe":
                    panels[0] = pe_init["wq"]
                    for mo in range(MO):
                        colsum(mo)
                else:
                    load_x(0, 0)
                    load_x(0, 1)
                    panels[0] = new_panel()
                    produce_half(panels[0], 0, 0, 0, 0)
                    produce_half(panels[0], 0, 0, 0, 1)
                    colsum(0)
                    load_x(1, 0)
                    load_x(1, 1)
                    produce_half(panels[0], 0, 0, 1, 0)
                    produce_half(panels[0], 0, 0, 1, 1)
                    colsum(1)
                    for mo in (2, 3):
                        load_x(mo, 0)
                        load_x(mo, 1)
                        produce_half(panels[0], 0, 0, mo, 0)
                        produce_half(panels[0], 0, 0, mo, 1)
                        colsum(mo)
                    for j in (4, 5, 6, 7):
                        produce_half(panels[0], 0, 0, j, 0)
                        produce_half(panels[0], 0, 0, j, 1)

                for p in range(NH * KB):
                    nh, kb = divmod(p, KB)
                    wq = panels.pop(p)
                    np_nh, np_kb = divmod(p + 1, KB)
                    for mo in range(MO):
                        ps = psum.tile([P, NW], F32, tag="ps", name="ps")
                        for j in range(KBO):
                            k = kb * KBO + j
                            for t in range(NW // 512):
                                nc.tensor.matmul(
                                    ps[:, t * 512:(t + 1) * 512],
                                    lhsT=xres[mo][:, k, :],
                                    rhs=wq[:, j, t * 512:(t + 1) * 512],
                                    start=(j == 0),
                                    stop=(j == KBO - 1),
                                )
                        # evict: acc += a/2 * psum (bf16); kb0 adds the
                        # colsum bias (split-quant); kb3 writes f32 out.
                        if kb == 0:
                            if quant == "split":
                                nc.vector.tensor_scalar(
                                    acc[:, mo, :], ps[:, :], half[:, :],
                                    nas[mo][:, :], MULT, ADD,
                                )
                            else:
                                nc.vector.tensor_scalar(
                                    acc[:, mo, :], ps[:, :], half[:, :],
                                    None, MULT,
                                )
                        elif kb < KB - 1:
                            nc.vector.scalar_tensor_tensor(
                                acc[:, mo, :], ps[:, :], half[:, :],
                                acc[:, mo, :], MULT, ADD,
                            )
                        else:
                            for h in range(2):
                                sl = slice(h * 1024, (h + 1) * 1024)
                                ot = otp.tile([P, 1024], F32, tag="ot",
                                              name="ot")
                                nc.vector.scalar_tensor_tensor(
                                    ot[:, :], ps[:, sl], half[:, :],
                                    acc[:, mo, sl], MULT, ADD,
                                )
                                nc.scalar.dma_start(
                                    out=out_d.ap()[:, mo,
                                                   nh * NW + h * 1024:
                                                   nh * NW + (h + 1) * 1024],
                                    in_=ot[:, :],
                                )
                        if mode != "pe":
                            if p == 0 and 0 <= mo < 4:
                                load_x(mo + 4, 0)
                                load_x(mo + 4, 1)
                                colsum(mo + 4)
                            if p + 1 < NH * KB:
                                if p + 1 not in panels:
                                    panels[p + 1] = new_panel()
                                produce_half(panels[p + 1], np_nh, np_kb,
                                             mo, 0)
                                produce_half(panels[p + 1], np_nh, np_kb,
                                             mo, 1)
                        elif p + 1 < NH * KB:
                            panels[p + 1] = pe_init["wq"]

    nc.compile()
    return nc


_NC_CACHE = {}


def _get_nc():
    if "nc" not in _NC_CACHE:
        _NC_CACHE["nc"] = build()
    return _NC_CACHE["nc"]


def make_in_maps(x, W, alpha):
    x = np.ascontiguousarray(np.asarray(x, np.float32)).reshape(M_TOT, D_IN)
    W = np.ascontiguousarray(np.asarray(W, np.float32))
    a = np.full((1, 1), np.float32(np.asarray(alpha)), np.float32)
    # wt[p, k, n] = W[n, k*128 + p]
    wt = np.ascontiguousarray(W.reshape(D_OUT, KO, P).transpose(2, 1, 0))
    in_maps = []
    for c in range(N_CORES):
        xs = x[c * M_SHARD:(c + 1) * M_SHARD]
        # xt[p, mo, k, m] = xs[mo*128 + m, k*128 + p]
        xt = np.ascontiguousarray(
            xs.reshape(MO, P, KO, P).transpose(3, 0, 2, 1))
        in_maps.append({"xt": xt, "wt": wt, "alpha": a})
    return in_maps


def gather_out(results):
    outs = []
    for c in range(N_CORES):
        o = results[c]["out"]  # [P, MO, D_OUT]; row = mo*128 + p
        outs.append(o.transpose(1, 0, 2).reshape(M_SHARD, D_OUT))
    return np.concatenate(outs, axis=0).reshape(4, 2048, D_OUT)


def kernel(x, W, alpha):
    nc = _get_nc()
    in_maps = make_in_maps(x, W, alpha)
    res = run_bass_kernel_spmd(nc, in_maps, core_ids=list(range(N_CORES)))
    return gather_out(res.results)
